# revision 23
# baseline (speedup 1.0000x reference)
"""Tensor-parallel GQA attention prefill for 8 TRN2 NeuronCores.

Sharding: each core owns 4 query heads + 1 kv head (column-shard of
wq/wk/wv by head) and a 512-row slice of wo's input dim (row-shard).
Each core computes a partial output projection over its local heads;
the host sums the 8 partials (the all-reduce) and transposes back.

Fast path (bf16), redesigned from NTFF trace analysis of the previous
schedule (107.4us; 71.0us of real matmul columns at 2.4GHz):
  - j-bundled streaming: DRAM stream sxw packs [x_j|wk_j|wv_j|wq0_j|
    wq1_j] per contraction tile j. One DMA per j (pairs after j1) on
    the sync queue delivers exactly what the 4-way bundled sweep
    (K,V,Q0,Q1 per j) consumes next: real matmuls start ~9us and the
    x stream can never starve the PE (wire 0.73us/j vs PE 0.87us/j).
  - Q2/Q3 sweeps follow from SBUF-resident x; attention chains for
    heads 0..2 (rope/scores/softmax/ptrans/pv) ride under them on
    ACT/DVE with their small PE ops inserted between sweep matmuls.
  - Q3 is swept in column halves (batches 01 then 23) so the last
    head's softmax chain starts ~3.6us before the sweep ends; its
    chain is also split in halves.
  - O-projection dtile groups accumulate heads 0..2 in PSUM during
    the attention tail; after oT3 lands only h3 matmuls + evictions
    remain. PSUM: 5-bank ring (sweeps then y dtiles) + 3-bank ring
    (warm/scores/ptrans/pv transients) = 8 banks exactly.
  - causal mask preloaded into the scores psum by an identity matmul
    and softmax denominators from ACT exp accum_out: no DVE mask-add
    or reduce on the attention chains.
  - 9 junk warmup matmuls lift the HAM clock gate and bridge the DMA
    ramp until the first bundle lands.
"""

import math
from contextlib import ExitStack

import ml_dtypes
import numpy as np

import concourse.bass as bass
import concourse.tile as tile
from concourse import bacc, mybir
from concourse.bass_utils import run_bass_kernel_spmd

DIM = 4096
N_HEADS = 32
HEAD_DIM = 128
N_KV_HEADS = 8
BSZ = 4
SEQLEN = 128
T = BSZ * SEQLEN  # 512 tokens
NCORES = 8
HQ = N_HEADS // NCORES  # 4 query heads per core
EQ = HQ * HEAD_DIM  # 512 local q features
ND = DIM // 128  # 32 contraction tiles
SCALE = 1.0 / math.sqrt(HEAD_DIM)
JW = T + 4 * HEAD_DIM  # 1024 cols per j-bundle: x | wk | wv | wq0 | wq1

F32 = mybir.dt.float32
F32R = mybir.dt.float32r
BF16 = mybir.dt.bfloat16
AX = mybir.AxisListType
ACTF = mybir.ActivationFunctionType
PSUM = bass.MemorySpace.PSUM


import os

USE_XROPE = os.environ.get("K_XROPE", "0") == "1"
USE_MASKMM = os.environ.get("K_MASKMM", "1") == "1"
USE_ACCEXP = os.environ.get("K_ACCEXP", "1") == "1"
USE_OHALF = os.environ.get("K_OHALF", "0") == "1"

_STATE: dict = {}
LAST_RESULT = None


def _install_ntff_hook():
    """Register the axon NTFF profile hook if the image lacks antenv.axon_hooks."""
    import os
    import sys
    import types

    try:
        import antenv.axon_hooks  # noqa: F401

        return
    except ImportError:
        pass
    try:
        import antenv
        from trn_agent_boot.trn_boot import _ntff_profile_via_ctypes

        mod = types.ModuleType("antenv.axon_hooks")
        holder = {"hook": None}
        mod.set_axon_ntff_profile_hook = lambda h: holder.__setitem__("hook", h)
        mod.get_axon_ntff_profile_hook = lambda: holder["hook"]
        sys.modules["antenv.axon_hooks"] = mod
        antenv.axon_hooks = mod
        so = "/opt/axon/libaxon_pjrt.so"
        if os.path.exists(so):
            hook = _ntff_profile_via_ctypes(so)
            if hook is not None:
                mod.set_axon_ntff_profile_hook(hook)
    except Exception:
        pass


_install_ntff_hook()


def _build_nc_fast():
    nc = bacc.Bacc(
        "TRN2",
        target_bir_lowering=False,
        debug=False,
        enable_asserts=False,
        num_devices=NCORES,
    )
    sxw = nc.dram_tensor("sxw", [128, ND * JW], BF16, kind="ExternalInput").ap()
    wq2T = nc.dram_tensor("wq2T", [128, ND * HEAD_DIM], BF16, kind="ExternalInput").ap()
    wq3T = nc.dram_tensor("wq3T", [128, ND * HEAD_DIM], BF16, kind="ExternalInput").ap()
    woT = nc.dram_tensor("woT", [128, HQ * DIM], BF16, kind="ExternalInput").ap()
    mask4 = nc.dram_tensor("mask4", [128, T], BF16, kind="ExternalInput").ap()
    cq = nc.dram_tensor("cq", [128, T], BF16, kind="ExternalInput").ap()
    sq = nc.dram_tensor("sq", [128, T], BF16, kind="ExternalInput").ap()
    ck = nc.dram_tensor("ck", [128, T], BF16, kind="ExternalInput").ap()
    sk = nc.dram_tensor("sk", [128, T], BF16, kind="ExternalInput").ap()
    ident = nc.dram_tensor("ident", [128, 128], BF16, kind="ExternalInput").ap()
    yT = nc.dram_tensor("yT", [DIM, T], BF16, kind="ExternalOutput").ap()

    with tile.TileContext(nc) as tc, ExitStack() as ctx:
        const = ctx.enter_context(tc.tile_pool(name="const", bufs=1))
        qtp = ctx.enter_context(tc.tile_pool(name="qtp", bufs=4))
        rt = ctx.enter_context(tc.tile_pool(name="rt", bufs=2))
        sm = ctx.enter_context(tc.tile_pool(name="sm", bufs=4))
        yp = ctx.enter_context(tc.tile_pool(name="yp", bufs=4))
        psA = ctx.enter_context(tc.tile_pool(name="psA", bufs=5, space=PSUM))
        psB = ctx.enter_context(tc.tile_pool(name="psB", bufs=3, space=PSUM))

        # ---- junk warmup: lift the HAM clock gate while the first bundle
        # lands (memset on DVE is its first post-barrier op) ----
        junk = const.tile([128, T], BF16, tag="junk")
        nc.vector.memset(junk[:], 0.0)
        ps_warm = psB.tile([128, T], F32, tag="ps", name="warm")
        for _ in range(9):
            nc.tensor.matmul(ps_warm[:], junk[:, 0:128], junk[:], start=True, stop=True)

        # ---- small constants via gpsimd SWDGE ----
        ident_sb = const.tile([128, 128], BF16, tag="ident")
        nc.gpsimd.dma_start(ident_sb[:], ident)
        ck_sb = const.tile([128, T], BF16, tag="ck")
        nc.gpsimd.dma_start(ck_sb[:], ck)
        sk_sb = const.tile([128, T], BF16, tag="sk")
        nc.gpsimd.dma_start(sk_sb[:], sk)
        cq_sb = const.tile([128, T], BF16, tag="cq")
        nc.gpsimd.dma_start(cq_sb[:], cq)
        sq_sb = const.tile([128, T], BF16, tag="sq")
        nc.gpsimd.dma_start(sq_sb[:], sq)
        mask_sb = const.tile([128, T], BF16, tag="mask4")
        nc.gpsimd.dma_start(mask_sb[:], mask4)

        # ---- ALL big inputs on the sync HWDGE queue in need-order (a
        # second queue would steal DMA-engine bandwidth from the critical
        # sxw stream): j0, j1 single (fast first data), then pairs; wq2
        # slotted before the last pair; wo last ----
        sxw_sb = const.tile([128, ND * JW], BF16, tag="sxw")
        wq2_sb = const.tile([128, ND * HEAD_DIM], BF16, tag="wq2")
        wq3_sb = const.tile([128, ND * HEAD_DIM], BF16, tag="wq3")
        wo_sb = const.tile([128, HQ * DIM], BF16, tag="wo")
        for j in range(4):
            nc.sync.dma_start(sxw_sb[:, j * JW : (j + 1) * JW], sxw[:, j * JW : (j + 1) * JW])
        for k in range(2, ND // 2 - 1):
            cs = slice(2 * k * JW, (2 * k + 2) * JW)
            nc.sync.dma_start(sxw_sb[:, cs], sxw[:, cs])
        nc.sync.dma_start(wq2_sb[:], wq2T)
        cs = slice((ND - 2) * JW, ND * JW)
        nc.sync.dma_start(sxw_sb[:, cs], sxw[:, cs])
        nc.sync.dma_start(wq3_sb[:], wq3T)
        for hq in range(HQ):
            nc.sync.dma_start(
                wo_sb[:, hq * DIM : (hq + 1) * DIM], woT[:, hq * DIM : (hq + 1) * DIM]
            )

        kT_sb = const.tile([128, T], BF16, tag="kT")
        vT_sb = const.tile([128, T], BF16, tag="vT")
        v_sb = const.tile([128, BSZ * HEAD_DIM], BF16, tag="v")
        oT_sb = const.tile([128, HQ * T], BF16, tag="oT")

        def xs(j):
            return sxw_sb[:, j * JW : j * JW + T]

        def wj(j, which):  # 0=wk 1=wv 2=wq0 3=wq1
            c0 = j * JW + T + which * HEAD_DIM
            return sxw_sb[:, c0 : c0 + HEAD_DIM]

        def rope(dst_ap, pssrc, ctab, stab, cs=slice(0, T)):
            # evict once to bf16 (ACT); 4 DVE ops (the half-swap is folded
            # into two cross-partition muls). ctab/stab are the table TILES.
            w = cs.stop - cs.start
            qe = rt.tile([128, T], BF16, tag="qe")
            nc.scalar.copy(qe[:, :w], pssrc)
            swp = rt.tile([128, T], BF16, tag="swp")
            if USE_XROPE:
                nc.vector.tensor_mul(swp[0:64, :w], qe[64:128, :w], stab[0:64, cs])
                nc.vector.tensor_mul(swp[64:128, :w], qe[0:64, :w], stab[64:128, cs])
            else:
                nc.vector.tensor_copy(swp[0:64, :w], qe[64:128, :w])
                nc.vector.tensor_copy(swp[64:128, :w], qe[0:64, :w])
                nc.vector.tensor_mul(swp[:, :w], swp[:, :w], stab[:, cs])
            prod = rt.tile([128, T], BF16, tag="prod")
            nc.vector.tensor_mul(prod[:, :w], qe[:, :w], ctab[:, cs])
            nc.vector.tensor_add(dst_ap, prod[:, :w], swp[:, :w])

        # ---- phase 1: 4-way bundled K/V/Q0/Q1 sweep riding the stream ----
        ps_k = psA.tile([128, T], F32, tag="ps", name="ps_k")
        ps_v = psA.tile([128, T], F32, tag="ps", name="ps_v")
        ps_q0 = psA.tile([128, T], F32, tag="ps", name="ps_q0")
        ps_q1 = psA.tile([128, T], F32, tag="ps", name="ps_q1")
        for j in range(ND):
            st, sp = (j == 0), (j == ND - 1)
            xr = xs(j)
            nc.tensor.matmul(ps_k[:], wj(j, 0), xr, start=st, stop=sp)
            nc.tensor.matmul(ps_v[:], wj(j, 1), xr, start=st, stop=sp)
            nc.tensor.matmul(ps_q0[:], wj(j, 2), xr, start=st, stop=sp)
            nc.tensor.matmul(ps_q1[:], wj(j, 3), xr, start=st, stop=sp)

        # vT evict on DVE so the ACT rope-evict chain starts immediately
        nc.vector.tensor_copy(vT_sb[:], ps_v[:])
        rope(kT_sb[:], ps_k[:], ck_sb, sk_sb)
        qts = {}
        qts[0] = qtp.tile([128, T], BF16, tag="qT", name="qT0")
        rope(qts[0][:], ps_q0[:], cq_sb, sq_sb)
        qts[1] = qtp.tile([128, T], BF16, tag="qT", name="qT1")
        rope(qts[1][:], ps_q1[:], cq_sb, sq_sb)

        # small PE helpers -------------------------------------------------
        def scores(h, qt, cols=slice(0, T)):
            # the additive causal mask is PRELOADED into the psum bank by an
            # identity matmul (start=True); qk matmuls accumulate on top —
            # no DVE mask-add, and exp reads psum directly
            b0, b1 = cols.start // 128, cols.stop // 128
            ps_s = psB.tile([128, T], F32, tag="ps", name=f"ps_s{h}b{b0}")
            if USE_MASKMM:
                for b in range(b0, b1):
                    bs = slice(b * 128, (b + 1) * 128)
                    nc.tensor.matmul(
                        ps_s[:, bs], ident_sb[:], mask_sb[:, bs], start=True, stop=False
                    )
                    nc.tensor.matmul(
                        ps_s[:, bs], qt[:, bs], kT_sb[:, bs], start=False, stop=True
                    )
            else:
                for b in range(b0, b1):
                    bs = slice(b * 128, (b + 1) * 128)
                    nc.tensor.matmul(
                        ps_s[:, bs], qt[:, bs], kT_sb[:, bs], start=True, stop=True
                    )
            return ps_s

        def softmax(h, ps_s, cols=slice(0, T), acc=True):
            # no max-subtract: fast path gates score sigma < 8. Per-batch
            # exp with free-axis accumulation gives the denominator on ACT
            # (no DVE reduce).
            b0, b1 = cols.start // 128, cols.stop // 128
            nb = b1 - b0
            p_sb = sm.tile([128, T], BF16, tag="p", name=f"p{h}b{b0}")
            den = sm.tile([128, BSZ], F32, tag="den", name=f"den{h}b{b0}")
            if acc and USE_ACCEXP:
                for b in range(b0, b1):
                    bs = slice(b * 128, (b + 1) * 128)
                    nc.scalar.activation(
                        p_sb[:, bs],
                        ps_s[:, bs],
                        ACTF.Exp,
                        accum_out=den[:, b - b0 : b - b0 + 1],
                    )
            else:
                s_sb = sm.tile([128, T], F32, tag="s", name=f"s{h}b{b0}")
                if USE_MASKMM:
                    nc.vector.tensor_copy(s_sb[:, cols], ps_s[:, cols])
                else:
                    nc.vector.tensor_add(s_sb[:, cols], ps_s[:, cols], mask_sb[:, cols])
                nc.scalar.activation(p_sb[:, cols], s_sb[:, cols], ACTF.Exp)
            p3 = p_sb[:, cols].rearrange("p (b k) -> p b k", b=nb)
            if not (acc and USE_ACCEXP):
                nc.vector.reduce_sum(den[:, :nb], p3, axis=AX.X)
            rden = sm.tile([128, BSZ], BF16, tag="rden", name=f"rden{h}b{b0}")
            with nc.allow_low_precision(reason="1/den at 0.4% rel err is fine"):
                nc.vector.reciprocal(rden[:, :nb], den[:, :nb])
            rb = rden[:, :nb].unsqueeze(2).broadcast_to([128, nb, 128])
            nc.vector.tensor_mul(p3, p3, rb)
            return p_sb

        def ptrans(h, p_sb, cols=slice(0, T)):
            b0, b1 = cols.start // 128, cols.stop // 128
            ps_pt = psB.tile([128, T], BF16, tag="ps", name=f"ps_pt{h}b{b0}")
            for b in range(b0, b1):
                bs = slice(b * 128, (b + 1) * 128)
                nc.tensor.transpose(ps_pt[:, bs], p_sb[:, bs], ident_sb[:])
            pt_sb = sm.tile([128, T], BF16, tag="pt", name=f"pt{h}b{b0}")
            nc.scalar.copy(pt_sb[:, cols], ps_pt[:, cols])
            return pt_sb

        def pv(h, pt_sb, cols=slice(0, T)):
            b0, b1 = cols.start // 128, cols.stop // 128
            ps_o = psB.tile([128, T], F32, tag="ps", name=f"ps_o{h}b{b0}")
            for b in range(b0, b1):
                bs = slice(b * 128, (b + 1) * 128)
                nc.tensor.matmul(
                    ps_o[:, bs], v_sb[:, bs], pt_sb[:, bs], start=True, stop=True
                )
            nc.vector.tensor_copy(
                oT_sb[:, h * T + cols.start : h * T + cols.stop], ps_o[:, cols]
            )

        # ---- phase 2: Q2 sweep with V-transpose + scores0 inserted ----
        ps_q2 = psA.tile([128, T], F32, tag="ps", name="ps_q2")
        for j in range(ND):
            st, sp = (j == 0), (j == ND - 1)
            nc.tensor.matmul(
                ps_q2[:], wq2_sb[:, j * 128 : (j + 1) * 128], xs(j), start=st, stop=sp
            )
            if j == 5:
                # V transpose: vT evicted by ACT right after phase 1
                ps_vt = psB.tile([128, T], BF16, tag="ps", name="ps_vt")
                for b in range(BSZ):
                    bs = slice(b * 128, (b + 1) * 128)
                    nc.tensor.transpose(ps_vt[:, bs], vT_sb[:, bs], ident_sb[:])
            elif j == 18:
                ps_s0 = scores(0, qts[0])
                sm0 = softmax(0, ps_s0)
        nc.vector.tensor_copy(v_sb[:], ps_vt[:])  # DVE, after ropes 0/1
        qts[2] = qtp.tile([128, T], BF16, tag="qT", name="qT2")
        rope(qts[2][:], ps_q2[:], cq_sb, sq_sb)

        # ---- phase 3: Q3 sweep in column halves (batches 01 then 23);
        # heads 0/1 attention PE ops inserted between sweep matmuls ----
        ps_q3 = psA.tile([128, T], F32, tag="ps", name="ps_q3")
        HA, HB = slice(0, T // 2), slice(T // 2, T)
        for j in range(ND):
            st, sp = (j == 0), (j == ND - 1)
            nc.tensor.matmul(
                ps_q3[:, HA],
                wq3_sb[:, j * 128 : (j + 1) * 128],
                xs(j)[:, HA],
                start=st,
                stop=sp,
            )
            if j == 2:
                pt0 = ptrans(0, sm0)
            elif j == 12:
                pv(0, pt0)
            elif j == 14:
                ps_s1 = scores(1, qts[1])
                sm1 = softmax(1, ps_s1)
        pt1 = ptrans(1, sm1)
        qt3 = qtp.tile([128, T], BF16, tag="qT", name="qT3")
        rope(qt3[:, HA], ps_q3[:, HA], cq_sb, sq_sb, HA)
        for j in range(ND):
            st, sp = (j == 0), (j == ND - 1)
            nc.tensor.matmul(
                ps_q3[:, HB],
                wq3_sb[:, j * 128 : (j + 1) * 128],
                xs(j)[:, HB],
                start=st,
                stop=sp,
            )
            if j == 4:
                pv(1, pt1)
            elif j == 6:
                ps_s2 = scores(2, qts[2])
                sm2 = softmax(2, ps_s2, acc=False)
            elif j == 26:
                ps_s3a = scores(3, qt3, HA)
                sm3a = softmax(3, ps_s3a, HA, acc=False)
            elif j == 28:
                pt2 = ptrans(2, sm2)

        # ---- tail: head-3 halves chain + O-projection filling ----
        ps_y = {}

        def o_mm(dts, h, cols=slice(0, T)):
            for dt in dts:
                if h == 0:
                    ps_y[dt] = psA.tile([128, T], F32, tag="ps", name=f"ps_y{dt}")
                nc.tensor.matmul(
                    ps_y[dt][:, cols],
                    wo_sb[:, h * DIM + dt * 128 : h * DIM + (dt + 1) * 128],
                    oT_sb[:, h * T + cols.start : h * T + cols.stop],
                    start=(h == 0),
                    stop=(h == HQ - 1),
                )

        def y_out(dt, last=False):
            y_sb = yp.tile([128, T], BF16, tag="y", name=f"y{dt}")
            if last:
                h = T // 2
                nc.vector.tensor_copy(y_sb[:, :h], ps_y[dt][:, :h])
                nc.scalar.copy(y_sb[:, h:], ps_y[dt][:, h:])
                nc.sync.dma_start(yT[dt * 128 : (dt + 1) * 128, :h], y_sb[:, :h])
                nc.sync.dma_start(yT[dt * 128 : (dt + 1) * 128, h:], y_sb[:, h:])
            else:
                if dt % 2 == 0:
                    nc.vector.tensor_copy(y_sb[:], ps_y[dt][:])
                else:
                    nc.scalar.copy(y_sb[:], ps_y[dt][:])
                nc.sync.dma_start(yT[dt * 128 : (dt + 1) * 128, :], y_sb[:])

        rope(qt3[:, HB], ps_q3[:, HB], cq_sb, sq_sb, HB)
        o_mm(range(0, 3), 0)
        o_mm(range(3, 5), 0)
        pv(2, pt2)
        o_mm(range(0, 3), 1)
        pt3a = ptrans(3, sm3a, HA)
        o_mm(range(3, 5), 1)
        pv(3, pt3a, HA)
        ps_s3b = scores(3, qt3, HB)
        sm3b = softmax(3, ps_s3b, HB, acc=False)
        o_mm(range(0, 3), 2)
        pt3b = ptrans(3, sm3b, HB)
        o_mm(range(3, 5), 2)
        pv(3, pt3b, HB)
        # oT3 complete: stream the rest; groups of 3 dtiles over the
        # 5-bank psA ring keep evictions off the critical path; last
        # dtile alone so its eviction starts early
        o_mm(range(0, 3), 3)
        for dt in range(0, 3):
            y_out(dt)
        o_mm(range(3, 5), 3)
        for dt in range(3, 5):
            y_out(dt)
        for g0 in range(5, ND, 3):
            dts = range(g0, min(g0 + 3, ND - 1))
            for h in range(HQ):
                o_mm(dts, h)
            for dt in dts:
                y_out(dt)
        for h in range(HQ):
            o_mm([ND - 1], h)
        y_out(ND - 1, last=True)

    nc.compile()
    return nc


def _build_nc_robust():
    """fp32r q/k path — robust when softmax logits are winner-take-all.

    Kept close to the original structure (max-subtracted softmax)."""
    XD = F32R
    QD = F32
    TD = F32
    nc = bacc.Bacc(
        "TRN2",
        target_bir_lowering=False,
        debug=False,
        enable_asserts=False,
        num_devices=NCORES,
    )
    xT = nc.dram_tensor("xT", [128, ND * T], XD, kind="ExternalInput").ap()
    wqT = nc.dram_tensor("wqT", [128, HQ * ND * HEAD_DIM], XD, kind="ExternalInput").ap()
    wkT = nc.dram_tensor("wkT", [128, ND * HEAD_DIM], XD, kind="ExternalInput").ap()
    wvT = nc.dram_tensor("wvT", [128, ND * HEAD_DIM], XD, kind="ExternalInput").ap()
    woT = nc.dram_tensor("woT", [128, HQ * DIM], BF16, kind="ExternalInput").ap()
    mask1 = nc.dram_tensor("mask1", [128, 128], F32, kind="ExternalInput").ap()
    cq = nc.dram_tensor("cq", [128, T], TD, kind="ExternalInput").ap()
    sq = nc.dram_tensor("sq", [128, T], TD, kind="ExternalInput").ap()
    ck = nc.dram_tensor("ck", [128, T], TD, kind="ExternalInput").ap()
    sk = nc.dram_tensor("sk", [128, T], TD, kind="ExternalInput").ap()
    ident = nc.dram_tensor("ident", [128, 128], BF16, kind="ExternalInput").ap()
    yT = nc.dram_tensor("yT", [DIM, T], BF16, kind="ExternalOutput").ap()

    with tile.TileContext(nc) as tc, ExitStack() as ctx:
        const = ctx.enter_context(tc.tile_pool(name="const", bufs=1))
        wp = ctx.enter_context(tc.tile_pool(name="wp", bufs=4))
        qtp = ctx.enter_context(tc.tile_pool(name="qtp", bufs=4))
        rt = ctx.enter_context(tc.tile_pool(name="rt", bufs=1))
        sm = ctx.enter_context(tc.tile_pool(name="sm", bufs=2))
        yp = ctx.enter_context(tc.tile_pool(name="yp", bufs=2))
        ps = ctx.enter_context(tc.tile_pool(name="ps", bufs=7, space=PSUM))
        wps = ctx.enter_context(tc.tile_pool(name="wps", bufs=1, space=PSUM))

        warm_w = const.tile([128, 128], BF16, tag="warm_w")
        nc.vector.memset(warm_w[:], 0.0)
        warm_x = const.tile([128, T], BF16, tag="warm_x")
        nc.vector.memset(warm_x[:], 0.0)
        ps_warm = wps.tile([128, T], F32, tag="wps")
        for _ in range(10):
            nc.tensor.matmul(ps_warm[:], warm_w[:], warm_x[:], start=True, stop=True)

        wk_sb = wp.tile([128, ND * HEAD_DIM], XD, tag="w", name="wk")
        nc.sync.dma_start(wk_sb[:], wkT)
        wv_sb = wp.tile([128, ND * HEAD_DIM], XD, tag="w", name="wv")
        nc.scalar.dma_start(wv_sb[:], wvT)

        XGROUPS = [2, 2, 2, 2, 4, 4, 4, 4, 4, 4]
        XG_COL = []
        _j0 = 0
        for _gd in XGROUPS:
            XG_COL.append((_j0, _gd))
            _j0 += _gd

        x_tiles = [None] * len(XGROUPS)

        def load_x(gi, eng):
            j0, gd = XG_COL[gi]
            xg = const.tile([128, gd * T], XD, tag=f"x{gi}", name=f"x{gi}")
            eng.dma_start(xg[:], xT[:, j0 * T : (j0 + gd) * T])
            x_tiles[gi] = xg

        wq_tiles = [None] * HQ

        def load_wq(h, eng):
            wqt = wp.tile([128, ND * HEAD_DIM], XD, tag="w", name=f"wq{h}")
            eng.dma_start(wqt[:], wqT[:, h * DIM : (h + 1) * DIM])
            wq_tiles[h] = wqt

        load_x(0, nc.sync)
        load_wq(0, nc.scalar)
        load_wq(1, nc.sync)
        load_x(1, nc.scalar)
        load_wq(2, nc.sync)
        load_x(2, nc.scalar)
        load_wq(3, nc.sync)
        ident_sb = const.tile([128, 128], BF16, tag="ident")
        nc.scalar.dma_start(ident_sb[:], ident)
        ck_sb = const.tile([128, T], TD, tag="ck")
        nc.scalar.dma_start(ck_sb[:], ck)
        sk_sb = const.tile([128, T], TD, tag="sk")
        nc.scalar.dma_start(sk_sb[:], sk)
        cq_sb = const.tile([128, T], TD, tag="cq")
        nc.scalar.dma_start(cq_sb[:], cq)
        sq_sb = const.tile([128, T], TD, tag="sq")
        nc.scalar.dma_start(sq_sb[:], sq)
        mask_sb = const.tile([128, 128], F32, tag="mask")
        nc.scalar.dma_start(mask_sb[:], mask1)
        for gi in range(3, len(XGROUPS)):
            load_x(gi, nc.scalar if gi % 2 == 0 else nc.sync)
        wo_sb = const.tile([128, HQ * DIM], BF16, tag="wo")
        nc.sync.dma_start(wo_sb[:, : 2 * DIM], woT[:, : 2 * DIM])
        nc.scalar.dma_start(wo_sb[:, 2 * DIM :], woT[:, 2 * DIM :])

        kT_sb = const.tile([128, T], QD, tag="kT")
        vT_sb = const.tile([128, T], BF16, tag="vT")
        v_sb = const.tile([128, BSZ * HEAD_DIM], BF16, tag="v")
        oT_sb = const.tile([128, HQ * T], BF16, tag="oT")

        def xslice(j):
            for i, (jj0, gd) in enumerate(XG_COL):
                if jj0 <= j < jj0 + gd:
                    return x_tiles[i][:, (j - jj0) * T : (j - jj0 + 1) * T]
            raise AssertionError(j)

        def rope(dst_ap, pssrc, ctab, stab):
            swp = rt.tile([128, T], F32, tag="swp")
            nc.scalar.copy(swp[0:64, :], pssrc[64:128, :])
            nc.scalar.copy(swp[64:128, :], pssrc[0:64, :])
            prod = rt.tile([128, T], F32, tag="prod")
            nc.vector.tensor_mul(prod[:], pssrc[:], ctab)
            nc.vector.tensor_mul(swp[:], swp[:], stab)
            nc.vector.tensor_add(dst_ap, prod[:], swp[:])

        ps_k = ps.tile([128, T], F32, tag="ps")
        ps_v = ps.tile([128, T], F32, tag="ps")
        ps_q = [None] * HQ
        NSW = 2
        for h in range(NSW):
            ps_q[h] = ps.tile([128, T], F32, tag="ps", name=f"ps_q{h}")
        for j in range(ND):
            st, sp = (j == 0), (j == ND - 1)
            xr = xslice(j)
            js = slice(j * HEAD_DIM, (j + 1) * HEAD_DIM)
            nc.tensor.matmul(ps_k[:], wk_sb[:, js], xr, start=st, stop=sp)
            nc.tensor.matmul(ps_v[:], wv_sb[:, js], xr, start=st, stop=sp)
            for h in range(NSW):
                nc.tensor.matmul(ps_q[h][:], wq_tiles[h][:, js], xr, start=st, stop=sp)

        rope(kT_sb[:], ps_k[:], ck_sb, sk_sb)
        qts = {}
        for h in range(NSW):
            qts[h] = qtp.tile([128, T], QD, tag="qT", name=f"qT{h}")
            rope(qts[h][:], ps_q[h][:], cq_sb[:], sq_sb[:])

        def q_sweep(h):
            ps_qh = ps.tile([128, T], F32, tag="ps", name=f"ps_q{h}")
            for j in range(ND):
                st, sp = (j == 0), (j == ND - 1)
                js = slice(j * HEAD_DIM, (j + 1) * HEAD_DIM)
                nc.tensor.matmul(
                    ps_qh[:], wq_tiles[h][:, js], xslice(j), start=st, stop=sp
                )
            qt = qtp.tile([128, T], QD, tag="qT", name=f"qT{h}")
            rope(qt[:], ps_qh[:], cq_sb[:], sq_sb[:])
            return qt

        def keep_warm(n=2):
            for _ in range(n):
                nc.tensor.matmul(
                    ps_warm[:], warm_w[:], warm_x[:], start=True, stop=True
                )

        def att_scores(h, qt):
            ps_s = ps.tile([128, T], F32, tag="ps", name=f"ps_s{h}")
            for b in range(BSZ):
                bs = slice(b * 128, (b + 1) * 128)
                nc.tensor.matmul(
                    ps_s[:, bs], qt[:, bs], kT_sb[:, bs], start=True, stop=True
                )
            s_sb = sm.tile([128, T], F32, tag="s", name=f"s{h}")
            nmx = sm.tile([128, BSZ], F32, tag="nmx", name=f"nmx{h}")
            den = sm.tile([128, BSZ], F32, tag="den", name=f"den{h}")
            rden = sm.tile([128, BSZ], F32, tag="rden", name=f"rden{h}")
            p_sb = sm.tile([128, T], BF16, tag="p", name=f"p{h}")
            for b in range(BSZ):
                bs = slice(b * 128, (b + 1) * 128)
                nc.vector.tensor_add(s_sb[:, bs], ps_s[:, bs], mask_sb[:])
                nc.vector.reduce_max(
                    nmx[:, b : b + 1], s_sb[:, bs], axis=AX.X, negate=True
                )
                nc.scalar.activation(
                    p_sb[:, bs],
                    s_sb[:, bs],
                    ACTF.Exp,
                    bias=nmx[:, b : b + 1],
                    accum_out=den[:, b : b + 1],
                )
            nc.vector.reciprocal(rden[:], den[:])
            for b in range(BSZ):
                bs = slice(b * 128, (b + 1) * 128)
                nc.vector.tensor_scalar_mul(p_sb[:, bs], p_sb[:, bs], rden[:, b : b + 1])
            return p_sb

        def att_ptrans(h, p_sb):
            ps_pt = ps.tile([128, T], BF16, tag="ps", name=f"ps_pt{h}")
            for b in range(BSZ):
                bs = slice(b * 128, (b + 1) * 128)
                nc.tensor.transpose(ps_pt[:, bs], p_sb[:, bs], ident_sb[:])
            pt_sb = sm.tile([128, T], BF16, tag="pt", name=f"pt{h}")
            nc.scalar.copy(pt_sb[:], ps_pt[:])
            return pt_sb

        def att_pv(h, pt_sb):
            ps_o = ps.tile([128, T], F32, tag="ps", name=f"ps_o{h}")
            for b in range(BSZ):
                bs = slice(b * 128, (b + 1) * 128)
                nc.tensor.matmul(
                    ps_o[:, bs], v_sb[:, bs], pt_sb[:, bs], start=True, stop=True
                )
            if h % 2 == 0:
                nc.vector.tensor_copy(oT_sb[:, h * T : (h + 1) * T], ps_o[:])
            else:
                nc.scalar.copy(oT_sb[:, h * T : (h + 1) * T], ps_o[:])

        probs = {}
        qts[2] = q_sweep(2)
        nc.scalar.copy(vT_sb[:], ps_v[:])
        for b in range(BSZ):
            bs = slice(b * 128, (b + 1) * 128)
            ps_t = ps.tile([128, T], BF16, tag="ps")
            nc.tensor.transpose(ps_t[:, 0:128], vT_sb[:, bs], ident_sb[:])
            nc.vector.tensor_copy(v_sb[:, bs], ps_t[:, 0:128])
        probs[0] = att_scores(0, qts[0])
        probs[1] = att_scores(1, qts[1])
        qts[3] = q_sweep(3)
        att_pv(0, probs[0])
        probs[2] = att_scores(2, qts[2])
        att_pv(1, probs[1])
        keep_warm(2)
        probs[3] = att_scores(3, qts[3])
        att_pv(2, probs[2])
        keep_warm(2)
        att_pv(3, probs[3])

        for dt in range(ND):
            ps_y = ps.tile([128, T], F32, tag="ps", name=f"ps_y{dt}")
            for j in range(HQ):
                nc.tensor.matmul(
                    ps_y[:],
                    wo_sb[:, j * DIM + dt * 128 : j * DIM + (dt + 1) * 128],
                    oT_sb[:, j * T : (j + 1) * T],
                    start=(j == 0),
                    stop=(j == HQ - 1),
                )
            y_sb = yp.tile([128, T], BF16, tag="y", name=f"y{dt}")
            if dt % 2 == 0:
                nc.vector.tensor_copy(y_sb[:], ps_y[:])
                nc.sync.dma_start(yT[dt * 128 : (dt + 1) * 128, :], y_sb[:])
            else:
                nc.scalar.copy(y_sb[:], ps_y[:])
                nc.scalar.dma_start(yT[dt * 128 : (dt + 1) * 128, :], y_sb[:])

    nc.compile()
    return nc


def get_nc(fast: bool):
    key = "nc_fast" if fast else "nc_robust"
    if key not in _STATE:
        _STATE[key] = _build_nc_fast() if fast else _build_nc_robust()
    return _STATE[key]


def _prep_in_maps(x, wq, wk, wv, wo, freqs_cos, freqs_sin, mask, fast):
    f32 = np.float32
    bf16 = ml_dtypes.bfloat16
    xd = bf16 if fast else f32
    x = np.asarray(x, f32)
    wq = np.asarray(wq, f32)
    wk = np.asarray(wk, f32)
    wv = np.asarray(wv, f32)
    wo = np.asarray(wo, f32)
    fc = np.asarray(freqs_cos, f32)
    fs = np.asarray(freqs_sin, f32)
    mask = np.asarray(mask, f32)

    # even features first, then odd: (2i, 2i+1) pairs -> (i, i+64)
    perm = np.concatenate([np.arange(0, HEAD_DIM, 2), np.arange(1, HEAD_DIM, 2)])
    wqp = wq.reshape(N_HEADS, HEAD_DIM, DIM)[:, perm, :].reshape(DIM, DIM)
    wkp = wk.reshape(N_KV_HEADS, HEAD_DIM, DIM)[:, perm, :].reshape(
        N_KV_HEADS * HEAD_DIM, DIM
    )

    def sw_x(xmat):  # [T, DIM] -> [128, ND*T]
        return np.ascontiguousarray(
            xmat.T.reshape(ND, 128, T).transpose(1, 0, 2).reshape(128, ND * T)
        )

    def sw_w(wmat):  # [E(128), DIM] -> [128, ND*E]
        E = wmat.shape[0]
        return np.ascontiguousarray(
            wmat.T.reshape(ND, 128, E).transpose(1, 0, 2).reshape(128, ND * E)
        )

    xT = sw_x(x.reshape(T, DIM)).astype(xd)
    C0 = np.vstack([fc.T, fc.T])  # [128, 128]: row p -> cos[t, p % 64]
    S0 = np.vstack([-fs.T, fs.T])
    td = bf16 if fast else f32
    cq = np.ascontiguousarray(np.tile(C0 * SCALE, (1, BSZ))).astype(td)
    sq = np.ascontiguousarray(np.tile(S0 * SCALE, (1, BSZ))).astype(td)
    ck = np.ascontiguousarray(np.tile(C0, (1, BSZ))).astype(td)
    sk = np.ascontiguousarray(np.tile(S0, (1, BSZ))).astype(td)
    ident = np.eye(128, dtype=bf16)

    in_maps = []
    xj = xT.reshape(128, ND, T)
    for c in range(NCORES):
        qrows = slice(c * EQ, (c + 1) * EQ)
        krows = slice(c * HEAD_DIM, (c + 1) * HEAD_DIM)
        wq_heads = [
            sw_w(wqp[c * EQ + h * HEAD_DIM : c * EQ + (h + 1) * HEAD_DIM, :])
            for h in range(HQ)
        ]
        wo_sw = np.ascontiguousarray(
            wo[:, qrows].T.reshape(HQ, 128, DIM).transpose(1, 0, 2).reshape(128, HQ * DIM)
        ).astype(bf16)
        if fast:
            wkj = sw_w(wkp[krows, :]).reshape(128, ND, HEAD_DIM)
            wvj = sw_w(wv[krows, :]).reshape(128, ND, HEAD_DIM)
            wq0j = wq_heads[0].reshape(128, ND, HEAD_DIM)
            wq1j = wq_heads[1].reshape(128, ND, HEAD_DIM)
            sxw = np.ascontiguousarray(
                np.concatenate([xj, wkj, wvj, wq0j, wq1j], axis=2).reshape(
                    128, ND * JW
                )
            ).astype(xd)
            m = {
                "sxw": sxw,
                "wq2T": np.ascontiguousarray(wq_heads[2]).astype(xd),
                "wq3T": np.ascontiguousarray(wq_heads[3]).astype(xd),
                "woT": wo_sw,
                "cq": cq,
                "sq": sq,
                "ck": ck,
                "sk": sk,
                "ident": ident,
                "mask4": np.ascontiguousarray(np.tile(mask[0, 0], (1, BSZ))).astype(
                    bf16
                ),
            }
        else:
            m = {
                "xT": xT,
                "wqT": np.ascontiguousarray(np.concatenate(wq_heads, axis=1)).astype(
                    xd
                ),
                "wkT": sw_w(wkp[krows, :]).astype(xd),
                "wvT": sw_w(wv[krows, :]).astype(xd),
                "woT": wo_sw,
                "cq": cq,
                "sq": sq,
                "ck": ck,
                "sk": sk,
                "ident": ident,
                "mask1": np.ascontiguousarray(mask[0, 0]),
            }
        in_maps.append(m)
    return in_maps


def _pick_fast(x, wq):
    """bf16 q/k only when softmax logits are smooth (score sigma small)."""
    sx = float(np.asarray(x, np.float32).std())
    sw = float(np.asarray(wq, np.float32).std())
    sigma = sx * sw * math.sqrt(DIM * HEAD_DIM) * SCALE
    return sigma < 8.0


def kernel(
    x,
    wq,
    wk,
    wv,
    wo,
    cache_k,
    cache_v,
    freqs_cos,
    freqs_sin,
    mask,
    start_pos,
    *,
    trace=False,
    trace_kwargs=None,
):
    global LAST_RESULT
    sp = int(np.asarray(start_pos))
    assert sp == 0, f"kernel specialized for start_pos=0, got {sp}"

    fast = _pick_fast(x, wq)
    in_maps = _prep_in_maps(x, wq, wk, wv, wo, freqs_cos, freqs_sin, mask, fast)
    nc = get_nc(fast)
    res = run_bass_kernel_spmd(
        nc,
        in_maps,
        core_ids=list(range(NCORES)),
        trace=trace,
        **(trace_kwargs or {}),
    )
    LAST_RESULT = res
    acc = np.zeros((DIM, T), np.float32)
    for c in range(NCORES):
        acc += res.results[c]["yT"].astype(np.float32)
    return np.ascontiguousarray(acc.T).reshape(BSZ, SEQLEN, DIM)


# revision 24
# speedup vs baseline: 1.0285x; 1.0285x over previous
"""Tensor-parallel GQA attention prefill for 8 TRN2 NeuronCores.

Sharding: each core owns 4 query heads + 1 kv head (column-shard of
wq/wk/wv by head) and a 512-row slice of wo's input dim (row-shard).
Each core computes a partial output projection over its local heads;
the host sums the 8 partials (the all-reduce) and transposes back.

Fast path (bf16), redesigned from NTFF trace analysis of the previous
schedule (107.4us; 71.0us of real matmul columns at 2.4GHz):
  - j-bundled streaming: DRAM stream sxw packs [x_j|wk_j|wv_j|wq0_j|
    wq1_j] per contraction tile j. One DMA per j (pairs after j1) on
    the sync queue delivers exactly what the 4-way bundled sweep
    (K,V,Q0,Q1 per j) consumes next: real matmuls start ~9us and the
    x stream can never starve the PE (wire 0.73us/j vs PE 0.87us/j).
  - Q2/Q3 sweeps follow from SBUF-resident x; attention chains for
    heads 0..2 (rope/scores/softmax/ptrans/pv) ride under them on
    ACT/DVE with their small PE ops inserted between sweep matmuls.
  - Q3 is swept in column halves (batches 01 then 23) so the last
    head's softmax chain starts ~3.6us before the sweep ends; its
    chain is also split in halves.
  - O-projection dtile groups accumulate heads 0..2 in PSUM during
    the attention tail; after oT3 lands only h3 matmuls + evictions
    remain. PSUM: 5-bank ring (sweeps then y dtiles) + 3-bank ring
    (warm/scores/ptrans/pv transients) = 8 banks exactly.
  - causal mask preloaded into the scores psum by an identity matmul
    and softmax denominators from ACT exp accum_out: no DVE mask-add
    or reduce on the attention chains.
  - 9 junk warmup matmuls lift the HAM clock gate and bridge the DMA
    ramp until the first bundle lands.
"""

import math
from contextlib import ExitStack

import ml_dtypes
import numpy as np

import concourse.bass as bass
import concourse.tile as tile
from concourse import bacc, mybir
from concourse.bass_utils import run_bass_kernel_spmd

DIM = 4096
N_HEADS = 32
HEAD_DIM = 128
N_KV_HEADS = 8
BSZ = 4
SEQLEN = 128
T = BSZ * SEQLEN  # 512 tokens
NCORES = 8
HQ = N_HEADS // NCORES  # 4 query heads per core
EQ = HQ * HEAD_DIM  # 512 local q features
ND = DIM // 128  # 32 contraction tiles
SCALE = 1.0 / math.sqrt(HEAD_DIM)
JW = T + 4 * HEAD_DIM  # 1024 cols per j-bundle: x | wk | wv | wq0 | wq1

F32 = mybir.dt.float32
F32R = mybir.dt.float32r
BF16 = mybir.dt.bfloat16
AX = mybir.AxisListType
ACTF = mybir.ActivationFunctionType
PSUM = bass.MemorySpace.PSUM


import os

USE_XROPE = os.environ.get("K_XROPE", "0") == "1"
USE_MASKMM = os.environ.get("K_MASKMM", "1") == "1"
USE_ACCEXP = os.environ.get("K_ACCEXP", "1") == "1"
USE_OHALF = os.environ.get("K_OHALF", "0") == "1"

_STATE: dict = {}
LAST_RESULT = None


def _install_ntff_hook():
    """Register the axon NTFF profile hook if the image lacks antenv.axon_hooks."""
    import os
    import sys
    import types

    try:
        import antenv.axon_hooks  # noqa: F401

        return
    except ImportError:
        pass
    try:
        import antenv
        from trn_agent_boot.trn_boot import _ntff_profile_via_ctypes

        mod = types.ModuleType("antenv.axon_hooks")
        holder = {"hook": None}
        mod.set_axon_ntff_profile_hook = lambda h: holder.__setitem__("hook", h)
        mod.get_axon_ntff_profile_hook = lambda: holder["hook"]
        sys.modules["antenv.axon_hooks"] = mod
        antenv.axon_hooks = mod
        so = "/opt/axon/libaxon_pjrt.so"
        if os.path.exists(so):
            hook = _ntff_profile_via_ctypes(so)
            if hook is not None:
                mod.set_axon_ntff_profile_hook(hook)
    except Exception:
        pass


_install_ntff_hook()


def _build_nc_fast():
    nc = bacc.Bacc(
        "TRN2",
        target_bir_lowering=False,
        debug=False,
        enable_asserts=False,
        num_devices=NCORES,
    )
    sxw = nc.dram_tensor("sxw", [128, ND * JW], BF16, kind="ExternalInput").ap()
    wq2T = nc.dram_tensor("wq2T", [128, ND * HEAD_DIM], BF16, kind="ExternalInput").ap()
    wq3T = nc.dram_tensor("wq3T", [128, ND * HEAD_DIM], BF16, kind="ExternalInput").ap()
    woT = nc.dram_tensor("woT", [128, HQ * DIM], BF16, kind="ExternalInput").ap()
    mask4 = nc.dram_tensor("mask4", [128, T], BF16, kind="ExternalInput").ap()
    cq = nc.dram_tensor("cq", [128, T], BF16, kind="ExternalInput").ap()
    sq = nc.dram_tensor("sq", [128, T], BF16, kind="ExternalInput").ap()
    ck = nc.dram_tensor("ck", [128, T], BF16, kind="ExternalInput").ap()
    sk = nc.dram_tensor("sk", [128, T], BF16, kind="ExternalInput").ap()
    ident = nc.dram_tensor("ident", [128, 128], BF16, kind="ExternalInput").ap()
    yT = nc.dram_tensor("yT", [DIM, T], BF16, kind="ExternalOutput").ap()

    with tile.TileContext(nc) as tc, ExitStack() as ctx:
        const = ctx.enter_context(tc.tile_pool(name="const", bufs=1))
        qtp = ctx.enter_context(tc.tile_pool(name="qtp", bufs=4))
        rt = ctx.enter_context(tc.tile_pool(name="rt", bufs=2))
        sm = ctx.enter_context(tc.tile_pool(name="sm", bufs=4))
        yp = ctx.enter_context(tc.tile_pool(name="yp", bufs=4))
        psA = ctx.enter_context(tc.tile_pool(name="psA", bufs=5, space=PSUM))
        psB = ctx.enter_context(tc.tile_pool(name="psB", bufs=3, space=PSUM))

        # ---- junk warmup: lift the HAM clock gate while the first bundle
        # lands (memset on DVE is its first post-barrier op) ----
        junk = const.tile([128, T], BF16, tag="junk")
        nc.vector.memset(junk[:], 0.0)
        ps_warm = psB.tile([128, T], F32, tag="ps", name="warm")
        for _ in range(9):
            nc.tensor.matmul(ps_warm[:], junk[:, 0:128], junk[:], start=True, stop=True)

        # ---- small constants via gpsimd SWDGE ----
        ident_sb = const.tile([128, 128], BF16, tag="ident")
        nc.gpsimd.dma_start(ident_sb[:], ident)
        ck_sb = const.tile([128, T], BF16, tag="ck")
        nc.gpsimd.dma_start(ck_sb[:], ck)
        sk_sb = const.tile([128, T], BF16, tag="sk")
        nc.gpsimd.dma_start(sk_sb[:], sk)
        cq_sb = const.tile([128, T], BF16, tag="cq")
        nc.gpsimd.dma_start(cq_sb[:], cq)
        sq_sb = const.tile([128, T], BF16, tag="sq")
        nc.gpsimd.dma_start(sq_sb[:], sq)
        mask_sb = const.tile([128, T], BF16, tag="mask4")
        nc.gpsimd.dma_start(mask_sb[:], mask4)

        # ---- ALL big inputs on the sync HWDGE queue in need-order (a
        # second queue would steal DMA-engine bandwidth from the critical
        # sxw stream): j0, j1 single (fast first data), then pairs; wq2
        # slotted before the last pair; wo last ----
        sxw_sb = const.tile([128, ND * JW], BF16, tag="sxw")
        wq2_sb = const.tile([128, ND * HEAD_DIM], BF16, tag="wq2")
        wq3_sb = const.tile([128, ND * HEAD_DIM], BF16, tag="wq3")
        wo_sb = const.tile([128, HQ * DIM], BF16, tag="wo")
        for j in range(4):
            nc.sync.dma_start(sxw_sb[:, j * JW : (j + 1) * JW], sxw[:, j * JW : (j + 1) * JW])
        for k in range(2, ND // 2 - 1):
            cs = slice(2 * k * JW, (2 * k + 2) * JW)
            nc.sync.dma_start(sxw_sb[:, cs], sxw[:, cs])
        nc.sync.dma_start(wq2_sb[:], wq2T)
        cs = slice((ND - 2) * JW, ND * JW)
        nc.sync.dma_start(sxw_sb[:, cs], sxw[:, cs])
        nc.sync.dma_start(wq3_sb[:], wq3T)
        for hq in range(HQ):
            nc.sync.dma_start(
                wo_sb[:, hq * DIM : (hq + 1) * DIM], woT[:, hq * DIM : (hq + 1) * DIM]
            )

        kT_sb = const.tile([128, T], BF16, tag="kT")
        vT_sb = const.tile([128, T], BF16, tag="vT")
        v_sb = const.tile([128, BSZ * HEAD_DIM], BF16, tag="v")
        oT_sb = const.tile([128, HQ * T], BF16, tag="oT")

        def xs(j):
            return sxw_sb[:, j * JW : j * JW + T]

        def wj(j, which):  # 0=wk 1=wv 2=wq0 3=wq1
            c0 = j * JW + T + which * HEAD_DIM
            return sxw_sb[:, c0 : c0 + HEAD_DIM]

        def rope(dst_ap, pssrc, ctab, stab, cs=slice(0, T)):
            # evict once to bf16 (ACT); 4 DVE ops (the half-swap is folded
            # into two cross-partition muls). ctab/stab are the table TILES.
            w = cs.stop - cs.start
            qe = rt.tile([128, T], BF16, tag="qe")
            nc.scalar.copy(qe[:, :w], pssrc)
            swp = rt.tile([128, T], BF16, tag="swp")
            if USE_XROPE:
                nc.vector.tensor_mul(swp[0:64, :w], qe[64:128, :w], stab[0:64, cs])
                nc.vector.tensor_mul(swp[64:128, :w], qe[0:64, :w], stab[64:128, cs])
            else:
                nc.vector.tensor_copy(swp[0:64, :w], qe[64:128, :w])
                nc.vector.tensor_copy(swp[64:128, :w], qe[0:64, :w])
                nc.vector.tensor_mul(swp[:, :w], swp[:, :w], stab[:, cs])
            prod = rt.tile([128, T], BF16, tag="prod")
            nc.vector.tensor_mul(prod[:, :w], qe[:, :w], ctab[:, cs])
            nc.vector.tensor_add(dst_ap, prod[:, :w], swp[:, :w])

        # ---- phase 1: 4-way bundled K/V/Q0/Q1 sweep riding the stream ----
        ps_k = psA.tile([128, T], F32, tag="ps", name="ps_k")
        ps_v = psA.tile([128, T], F32, tag="ps", name="ps_v")
        ps_q0 = psA.tile([128, T], F32, tag="ps", name="ps_q0")
        ps_q1 = psA.tile([128, T], F32, tag="ps", name="ps_q1")
        for j in range(ND):
            st, sp = (j == 0), (j == ND - 1)
            xr = xs(j)
            nc.tensor.matmul(ps_k[:], wj(j, 0), xr, start=st, stop=sp)
            nc.tensor.matmul(ps_v[:], wj(j, 1), xr, start=st, stop=sp)
            nc.tensor.matmul(ps_q0[:], wj(j, 2), xr, start=st, stop=sp)
            nc.tensor.matmul(ps_q1[:], wj(j, 3), xr, start=st, stop=sp)

        # vT evict on DVE so the ACT rope-evict chain starts immediately
        nc.vector.tensor_copy(vT_sb[:], ps_v[:])
        rope(kT_sb[:], ps_k[:], ck_sb, sk_sb)
        qts = {}
        qts[0] = qtp.tile([128, T], BF16, tag="qT", name="qT0")
        rope(qts[0][:], ps_q0[:], cq_sb, sq_sb)
        qts[1] = qtp.tile([128, T], BF16, tag="qT", name="qT1")
        rope(qts[1][:], ps_q1[:], cq_sb, sq_sb)

        # small PE helpers -------------------------------------------------
        def scores(h, qt, cols=slice(0, T)):
            # the additive causal mask is PRELOADED into the psum bank by an
            # identity matmul (start=True); qk matmuls accumulate on top —
            # no DVE mask-add, and exp reads psum directly
            b0, b1 = cols.start // 128, cols.stop // 128
            ps_s = psB.tile([128, T], F32, tag="ps", name=f"ps_s{h}b{b0}")
            if USE_MASKMM:
                for b in range(b0, b1):
                    bs = slice(b * 128, (b + 1) * 128)
                    nc.tensor.matmul(
                        ps_s[:, bs], ident_sb[:], mask_sb[:, bs], start=True, stop=False
                    )
                    nc.tensor.matmul(
                        ps_s[:, bs], qt[:, bs], kT_sb[:, bs], start=False, stop=True
                    )
            else:
                for b in range(b0, b1):
                    bs = slice(b * 128, (b + 1) * 128)
                    nc.tensor.matmul(
                        ps_s[:, bs], qt[:, bs], kT_sb[:, bs], start=True, stop=True
                    )
            return ps_s

        def softmax(h, ps_s, cols=slice(0, T), acc=True):
            # no max-subtract: fast path gates score sigma < 8. Per-batch
            # exp with free-axis accumulation gives the denominator on ACT
            # (no DVE reduce).
            b0, b1 = cols.start // 128, cols.stop // 128
            nb = b1 - b0
            p_sb = sm.tile([128, T], BF16, tag="p", name=f"p{h}b{b0}")
            den = sm.tile([128, BSZ], F32, tag="den", name=f"den{h}b{b0}")
            if acc and USE_ACCEXP:
                for b in range(b0, b1):
                    bs = slice(b * 128, (b + 1) * 128)
                    nc.scalar.activation(
                        p_sb[:, bs],
                        ps_s[:, bs],
                        ACTF.Exp,
                        accum_out=den[:, b - b0 : b - b0 + 1],
                    )
            elif USE_MASKMM:
                # mask already in psum: exp reads psum directly
                nc.scalar.activation(p_sb[:, cols], ps_s[:, cols], ACTF.Exp)
            else:
                s_sb = sm.tile([128, T], F32, tag="s", name=f"s{h}b{b0}")
                nc.vector.tensor_add(s_sb[:, cols], ps_s[:, cols], mask_sb[:, cols])
                nc.scalar.activation(p_sb[:, cols], s_sb[:, cols], ACTF.Exp)
            p3 = p_sb[:, cols].rearrange("p (b k) -> p b k", b=nb)
            if not (acc and USE_ACCEXP):
                nc.vector.reduce_sum(den[:, :nb], p3, axis=AX.X)
            rden = sm.tile([128, BSZ], BF16, tag="rden", name=f"rden{h}b{b0}")
            with nc.allow_low_precision(reason="1/den at 0.4% rel err is fine"):
                nc.vector.reciprocal(rden[:, :nb], den[:, :nb])
            rb = rden[:, :nb].unsqueeze(2).broadcast_to([128, nb, 128])
            nc.vector.tensor_mul(p3, p3, rb)
            return p_sb

        def ptrans(h, p_sb, cols=slice(0, T)):
            b0, b1 = cols.start // 128, cols.stop // 128
            ps_pt = psB.tile([128, T], BF16, tag="ps", name=f"ps_pt{h}b{b0}")
            for b in range(b0, b1):
                bs = slice(b * 128, (b + 1) * 128)
                nc.tensor.transpose(ps_pt[:, bs], p_sb[:, bs], ident_sb[:])
            pt_sb = sm.tile([128, T], BF16, tag="pt", name=f"pt{h}b{b0}")
            nc.scalar.copy(pt_sb[:, cols], ps_pt[:, cols])
            return pt_sb

        def pv(h, pt_sb, cols=slice(0, T)):
            b0, b1 = cols.start // 128, cols.stop // 128
            ps_o = psB.tile([128, T], F32, tag="ps", name=f"ps_o{h}b{b0}")
            for b in range(b0, b1):
                bs = slice(b * 128, (b + 1) * 128)
                nc.tensor.matmul(
                    ps_o[:, bs], v_sb[:, bs], pt_sb[:, bs], start=True, stop=True
                )
            nc.vector.tensor_copy(
                oT_sb[:, h * T + cols.start : h * T + cols.stop], ps_o[:, cols]
            )

        # ---- phase 2: Q2 sweep with V-transpose + scores0 inserted ----
        ps_q2 = psA.tile([128, T], F32, tag="ps", name="ps_q2")
        for j in range(ND):
            st, sp = (j == 0), (j == ND - 1)
            nc.tensor.matmul(
                ps_q2[:], wq2_sb[:, j * 128 : (j + 1) * 128], xs(j), start=st, stop=sp
            )
            if j == 5:
                # V transpose: vT evicted by ACT right after phase 1
                ps_vt = psB.tile([128, T], BF16, tag="ps", name="ps_vt")
                for b in range(BSZ):
                    bs = slice(b * 128, (b + 1) * 128)
                    nc.tensor.transpose(ps_vt[:, bs], vT_sb[:, bs], ident_sb[:])
            elif j == 18:
                ps_s0 = scores(0, qts[0])
                sm0 = softmax(0, ps_s0)
        nc.vector.tensor_copy(v_sb[:], ps_vt[:])  # DVE, after ropes 0/1
        qts[2] = qtp.tile([128, T], BF16, tag="qT", name="qT2")
        rope(qts[2][:], ps_q2[:], cq_sb, sq_sb)

        # ---- phase 3: Q3 sweep in column halves (batches 01 then 23);
        # heads 0/1 attention PE ops inserted between sweep matmuls ----
        ps_q3 = psA.tile([128, T], F32, tag="ps", name="ps_q3")
        HA, HB = slice(0, T // 2), slice(T // 2, T)
        for j in range(ND):
            st, sp = (j == 0), (j == ND - 1)
            nc.tensor.matmul(
                ps_q3[:, HA],
                wq3_sb[:, j * 128 : (j + 1) * 128],
                xs(j)[:, HA],
                start=st,
                stop=sp,
            )
            if j == 2:
                pt0 = ptrans(0, sm0)
            elif j == 12:
                pv(0, pt0)
            elif j == 14:
                ps_s1 = scores(1, qts[1])
                sm1 = softmax(1, ps_s1)
        pt1 = ptrans(1, sm1)
        qt3 = qtp.tile([128, T], BF16, tag="qT", name="qT3")
        rope(qt3[:, HA], ps_q3[:, HA], cq_sb, sq_sb, HA)
        for j in range(ND):
            st, sp = (j == 0), (j == ND - 1)
            nc.tensor.matmul(
                ps_q3[:, HB],
                wq3_sb[:, j * 128 : (j + 1) * 128],
                xs(j)[:, HB],
                start=st,
                stop=sp,
            )
            if j == 4:
                pv(1, pt1)
            elif j == 6:
                ps_s2 = scores(2, qts[2])
                sm2 = softmax(2, ps_s2, acc=False)
            elif j == 26:
                ps_s3a = scores(3, qt3, HA)
                sm3a = softmax(3, ps_s3a, HA, acc=False)
            elif j == 28:
                pt2 = ptrans(2, sm2)

        # ---- tail: head-3 halves chain + O-projection filling ----
        ps_y = {}

        def o_mm(dts, h, cols=slice(0, T)):
            for dt in dts:
                if h == 0:
                    ps_y[dt] = psA.tile([128, T], F32, tag="ps", name=f"ps_y{dt}")
                nc.tensor.matmul(
                    ps_y[dt][:, cols],
                    wo_sb[:, h * DIM + dt * 128 : h * DIM + (dt + 1) * 128],
                    oT_sb[:, h * T + cols.start : h * T + cols.stop],
                    start=(h == 0),
                    stop=(h == HQ - 1),
                )

        def y_out(dt, last=False):
            y_sb = yp.tile([128, T], BF16, tag="y", name=f"y{dt}")
            if last:
                h = T // 2
                nc.vector.tensor_copy(y_sb[:, :h], ps_y[dt][:, :h])
                nc.scalar.copy(y_sb[:, h:], ps_y[dt][:, h:])
                nc.sync.dma_start(yT[dt * 128 : (dt + 1) * 128, :h], y_sb[:, :h])
                nc.sync.dma_start(yT[dt * 128 : (dt + 1) * 128, h:], y_sb[:, h:])
            else:
                if dt % 2 == 0:
                    nc.vector.tensor_copy(y_sb[:], ps_y[dt][:])
                else:
                    nc.scalar.copy(y_sb[:], ps_y[dt][:])
                nc.sync.dma_start(yT[dt * 128 : (dt + 1) * 128, :], y_sb[:])

        rope(qt3[:, HB], ps_q3[:, HB], cq_sb, sq_sb, HB)
        o_mm(range(0, 3), 0)
        o_mm(range(3, 5), 0)
        pv(2, pt2)
        o_mm(range(0, 3), 1)
        pt3a = ptrans(3, sm3a, HA)
        o_mm(range(3, 5), 1)
        pv(3, pt3a, HA)
        ps_s3b = scores(3, qt3, HB)
        sm3b = softmax(3, ps_s3b, HB, acc=False)
        o_mm(range(0, 3), 2)
        pt3b = ptrans(3, sm3b, HB)
        o_mm(range(3, 5), 2)
        pv(3, pt3b, HB)
        # oT3 complete: stream the rest; groups of 3 dtiles over the
        # 5-bank psA ring keep evictions off the critical path; last
        # dtile alone so its eviction starts early
        o_mm(range(0, 3), 3)
        for dt in range(0, 3):
            y_out(dt)
        o_mm(range(3, 5), 3)
        for dt in range(3, 5):
            y_out(dt)
        for g0 in range(5, ND, 3):
            dts = range(g0, min(g0 + 3, ND - 1))
            for h in range(HQ):
                o_mm(dts, h)
            for dt in dts:
                y_out(dt)
        for h in range(HQ):
            o_mm([ND - 1], h)
        y_out(ND - 1, last=True)

    nc.compile()
    return nc


def _build_nc_robust():
    """fp32r q/k path — robust when softmax logits are winner-take-all.

    Kept close to the original structure (max-subtracted softmax)."""
    XD = F32R
    QD = F32
    TD = F32
    nc = bacc.Bacc(
        "TRN2",
        target_bir_lowering=False,
        debug=False,
        enable_asserts=False,
        num_devices=NCORES,
    )
    xT = nc.dram_tensor("xT", [128, ND * T], XD, kind="ExternalInput").ap()
    wqT = nc.dram_tensor("wqT", [128, HQ * ND * HEAD_DIM], XD, kind="ExternalInput").ap()
    wkT = nc.dram_tensor("wkT", [128, ND * HEAD_DIM], XD, kind="ExternalInput").ap()
    wvT = nc.dram_tensor("wvT", [128, ND * HEAD_DIM], XD, kind="ExternalInput").ap()
    woT = nc.dram_tensor("woT", [128, HQ * DIM], BF16, kind="ExternalInput").ap()
    mask1 = nc.dram_tensor("mask1", [128, 128], F32, kind="ExternalInput").ap()
    cq = nc.dram_tensor("cq", [128, T], TD, kind="ExternalInput").ap()
    sq = nc.dram_tensor("sq", [128, T], TD, kind="ExternalInput").ap()
    ck = nc.dram_tensor("ck", [128, T], TD, kind="ExternalInput").ap()
    sk = nc.dram_tensor("sk", [128, T], TD, kind="ExternalInput").ap()
    ident = nc.dram_tensor("ident", [128, 128], BF16, kind="ExternalInput").ap()
    yT = nc.dram_tensor("yT", [DIM, T], BF16, kind="ExternalOutput").ap()

    with tile.TileContext(nc) as tc, ExitStack() as ctx:
        const = ctx.enter_context(tc.tile_pool(name="const", bufs=1))
        wp = ctx.enter_context(tc.tile_pool(name="wp", bufs=4))
        qtp = ctx.enter_context(tc.tile_pool(name="qtp", bufs=4))
        rt = ctx.enter_context(tc.tile_pool(name="rt", bufs=1))
        sm = ctx.enter_context(tc.tile_pool(name="sm", bufs=2))
        yp = ctx.enter_context(tc.tile_pool(name="yp", bufs=2))
        ps = ctx.enter_context(tc.tile_pool(name="ps", bufs=7, space=PSUM))
        wps = ctx.enter_context(tc.tile_pool(name="wps", bufs=1, space=PSUM))

        warm_w = const.tile([128, 128], BF16, tag="warm_w")
        nc.vector.memset(warm_w[:], 0.0)
        warm_x = const.tile([128, T], BF16, tag="warm_x")
        nc.vector.memset(warm_x[:], 0.0)
        ps_warm = wps.tile([128, T], F32, tag="wps")
        for _ in range(10):
            nc.tensor.matmul(ps_warm[:], warm_w[:], warm_x[:], start=True, stop=True)

        wk_sb = wp.tile([128, ND * HEAD_DIM], XD, tag="w", name="wk")
        nc.sync.dma_start(wk_sb[:], wkT)
        wv_sb = wp.tile([128, ND * HEAD_DIM], XD, tag="w", name="wv")
        nc.scalar.dma_start(wv_sb[:], wvT)

        XGROUPS = [2, 2, 2, 2, 4, 4, 4, 4, 4, 4]
        XG_COL = []
        _j0 = 0
        for _gd in XGROUPS:
            XG_COL.append((_j0, _gd))
            _j0 += _gd

        x_tiles = [None] * len(XGROUPS)

        def load_x(gi, eng):
            j0, gd = XG_COL[gi]
            xg = const.tile([128, gd * T], XD, tag=f"x{gi}", name=f"x{gi}")
            eng.dma_start(xg[:], xT[:, j0 * T : (j0 + gd) * T])
            x_tiles[gi] = xg

        wq_tiles = [None] * HQ

        def load_wq(h, eng):
            wqt = wp.tile([128, ND * HEAD_DIM], XD, tag="w", name=f"wq{h}")
            eng.dma_start(wqt[:], wqT[:, h * DIM : (h + 1) * DIM])
            wq_tiles[h] = wqt

        load_x(0, nc.sync)
        load_wq(0, nc.scalar)
        load_wq(1, nc.sync)
        load_x(1, nc.scalar)
        load_wq(2, nc.sync)
        load_x(2, nc.scalar)
        load_wq(3, nc.sync)
        ident_sb = const.tile([128, 128], BF16, tag="ident")
        nc.scalar.dma_start(ident_sb[:], ident)
        ck_sb = const.tile([128, T], TD, tag="ck")
        nc.scalar.dma_start(ck_sb[:], ck)
        sk_sb = const.tile([128, T], TD, tag="sk")
        nc.scalar.dma_start(sk_sb[:], sk)
        cq_sb = const.tile([128, T], TD, tag="cq")
        nc.scalar.dma_start(cq_sb[:], cq)
        sq_sb = const.tile([128, T], TD, tag="sq")
        nc.scalar.dma_start(sq_sb[:], sq)
        mask_sb = const.tile([128, 128], F32, tag="mask")
        nc.scalar.dma_start(mask_sb[:], mask1)
        for gi in range(3, len(XGROUPS)):
            load_x(gi, nc.scalar if gi % 2 == 0 else nc.sync)
        wo_sb = const.tile([128, HQ * DIM], BF16, tag="wo")
        nc.sync.dma_start(wo_sb[:, : 2 * DIM], woT[:, : 2 * DIM])
        nc.scalar.dma_start(wo_sb[:, 2 * DIM :], woT[:, 2 * DIM :])

        kT_sb = const.tile([128, T], QD, tag="kT")
        vT_sb = const.tile([128, T], BF16, tag="vT")
        v_sb = const.tile([128, BSZ * HEAD_DIM], BF16, tag="v")
        oT_sb = const.tile([128, HQ * T], BF16, tag="oT")

        def xslice(j):
            for i, (jj0, gd) in enumerate(XG_COL):
                if jj0 <= j < jj0 + gd:
                    return x_tiles[i][:, (j - jj0) * T : (j - jj0 + 1) * T]
            raise AssertionError(j)

        def rope(dst_ap, pssrc, ctab, stab):
            swp = rt.tile([128, T], F32, tag="swp")
            nc.scalar.copy(swp[0:64, :], pssrc[64:128, :])
            nc.scalar.copy(swp[64:128, :], pssrc[0:64, :])
            prod = rt.tile([128, T], F32, tag="prod")
            nc.vector.tensor_mul(prod[:], pssrc[:], ctab)
            nc.vector.tensor_mul(swp[:], swp[:], stab)
            nc.vector.tensor_add(dst_ap, prod[:], swp[:])

        ps_k = ps.tile([128, T], F32, tag="ps")
        ps_v = ps.tile([128, T], F32, tag="ps")
        ps_q = [None] * HQ
        NSW = 2
        for h in range(NSW):
            ps_q[h] = ps.tile([128, T], F32, tag="ps", name=f"ps_q{h}")
        for j in range(ND):
            st, sp = (j == 0), (j == ND - 1)
            xr = xslice(j)
            js = slice(j * HEAD_DIM, (j + 1) * HEAD_DIM)
            nc.tensor.matmul(ps_k[:], wk_sb[:, js], xr, start=st, stop=sp)
            nc.tensor.matmul(ps_v[:], wv_sb[:, js], xr, start=st, stop=sp)
            for h in range(NSW):
                nc.tensor.matmul(ps_q[h][:], wq_tiles[h][:, js], xr, start=st, stop=sp)

        rope(kT_sb[:], ps_k[:], ck_sb, sk_sb)
        qts = {}
        for h in range(NSW):
            qts[h] = qtp.tile([128, T], QD, tag="qT", name=f"qT{h}")
            rope(qts[h][:], ps_q[h][:], cq_sb[:], sq_sb[:])

        def q_sweep(h):
            ps_qh = ps.tile([128, T], F32, tag="ps", name=f"ps_q{h}")
            for j in range(ND):
                st, sp = (j == 0), (j == ND - 1)
                js = slice(j * HEAD_DIM, (j + 1) * HEAD_DIM)
                nc.tensor.matmul(
                    ps_qh[:], wq_tiles[h][:, js], xslice(j), start=st, stop=sp
                )
            qt = qtp.tile([128, T], QD, tag="qT", name=f"qT{h}")
            rope(qt[:], ps_qh[:], cq_sb[:], sq_sb[:])
            return qt

        def keep_warm(n=2):
            for _ in range(n):
                nc.tensor.matmul(
                    ps_warm[:], warm_w[:], warm_x[:], start=True, stop=True
                )

        def att_scores(h, qt):
            ps_s = ps.tile([128, T], F32, tag="ps", name=f"ps_s{h}")
            for b in range(BSZ):
                bs = slice(b * 128, (b + 1) * 128)
                nc.tensor.matmul(
                    ps_s[:, bs], qt[:, bs], kT_sb[:, bs], start=True, stop=True
                )
            s_sb = sm.tile([128, T], F32, tag="s", name=f"s{h}")
            nmx = sm.tile([128, BSZ], F32, tag="nmx", name=f"nmx{h}")
            den = sm.tile([128, BSZ], F32, tag="den", name=f"den{h}")
            rden = sm.tile([128, BSZ], F32, tag="rden", name=f"rden{h}")
            p_sb = sm.tile([128, T], BF16, tag="p", name=f"p{h}")
            for b in range(BSZ):
                bs = slice(b * 128, (b + 1) * 128)
                nc.vector.tensor_add(s_sb[:, bs], ps_s[:, bs], mask_sb[:])
                nc.vector.reduce_max(
                    nmx[:, b : b + 1], s_sb[:, bs], axis=AX.X, negate=True
                )
                nc.scalar.activation(
                    p_sb[:, bs],
                    s_sb[:, bs],
                    ACTF.Exp,
                    bias=nmx[:, b : b + 1],
                    accum_out=den[:, b : b + 1],
                )
            nc.vector.reciprocal(rden[:], den[:])
            for b in range(BSZ):
                bs = slice(b * 128, (b + 1) * 128)
                nc.vector.tensor_scalar_mul(p_sb[:, bs], p_sb[:, bs], rden[:, b : b + 1])
            return p_sb

        def att_ptrans(h, p_sb):
            ps_pt = ps.tile([128, T], BF16, tag="ps", name=f"ps_pt{h}")
            for b in range(BSZ):
                bs = slice(b * 128, (b + 1) * 128)
                nc.tensor.transpose(ps_pt[:, bs], p_sb[:, bs], ident_sb[:])
            pt_sb = sm.tile([128, T], BF16, tag="pt", name=f"pt{h}")
            nc.scalar.copy(pt_sb[:], ps_pt[:])
            return pt_sb

        def att_pv(h, pt_sb):
            ps_o = ps.tile([128, T], F32, tag="ps", name=f"ps_o{h}")
            for b in range(BSZ):
                bs = slice(b * 128, (b + 1) * 128)
                nc.tensor.matmul(
                    ps_o[:, bs], v_sb[:, bs], pt_sb[:, bs], start=True, stop=True
                )
            if h % 2 == 0:
                nc.vector.tensor_copy(oT_sb[:, h * T : (h + 1) * T], ps_o[:])
            else:
                nc.scalar.copy(oT_sb[:, h * T : (h + 1) * T], ps_o[:])

        probs = {}
        qts[2] = q_sweep(2)
        nc.scalar.copy(vT_sb[:], ps_v[:])
        for b in range(BSZ):
            bs = slice(b * 128, (b + 1) * 128)
            ps_t = ps.tile([128, T], BF16, tag="ps")
            nc.tensor.transpose(ps_t[:, 0:128], vT_sb[:, bs], ident_sb[:])
            nc.vector.tensor_copy(v_sb[:, bs], ps_t[:, 0:128])
        probs[0] = att_scores(0, qts[0])
        probs[1] = att_scores(1, qts[1])
        qts[3] = q_sweep(3)
        att_pv(0, probs[0])
        probs[2] = att_scores(2, qts[2])
        att_pv(1, probs[1])
        keep_warm(2)
        probs[3] = att_scores(3, qts[3])
        att_pv(2, probs[2])
        keep_warm(2)
        att_pv(3, probs[3])

        for dt in range(ND):
            ps_y = ps.tile([128, T], F32, tag="ps", name=f"ps_y{dt}")
            for j in range(HQ):
                nc.tensor.matmul(
                    ps_y[:],
                    wo_sb[:, j * DIM + dt * 128 : j * DIM + (dt + 1) * 128],
                    oT_sb[:, j * T : (j + 1) * T],
                    start=(j == 0),
                    stop=(j == HQ - 1),
                )
            y_sb = yp.tile([128, T], BF16, tag="y", name=f"y{dt}")
            if dt % 2 == 0:
                nc.vector.tensor_copy(y_sb[:], ps_y[:])
                nc.sync.dma_start(yT[dt * 128 : (dt + 1) * 128, :], y_sb[:])
            else:
                nc.scalar.copy(y_sb[:], ps_y[:])
                nc.scalar.dma_start(yT[dt * 128 : (dt + 1) * 128, :], y_sb[:])

    nc.compile()
    return nc


def get_nc(fast: bool):
    key = "nc_fast" if fast else "nc_robust"
    if key not in _STATE:
        _STATE[key] = _build_nc_fast() if fast else _build_nc_robust()
    return _STATE[key]


def _prep_in_maps(x, wq, wk, wv, wo, freqs_cos, freqs_sin, mask, fast):
    f32 = np.float32
    bf16 = ml_dtypes.bfloat16
    xd = bf16 if fast else f32
    x = np.asarray(x, f32)
    wq = np.asarray(wq, f32)
    wk = np.asarray(wk, f32)
    wv = np.asarray(wv, f32)
    wo = np.asarray(wo, f32)
    fc = np.asarray(freqs_cos, f32)
    fs = np.asarray(freqs_sin, f32)
    mask = np.asarray(mask, f32)

    # even features first, then odd: (2i, 2i+1) pairs -> (i, i+64)
    perm = np.concatenate([np.arange(0, HEAD_DIM, 2), np.arange(1, HEAD_DIM, 2)])
    wqp = wq.reshape(N_HEADS, HEAD_DIM, DIM)[:, perm, :].reshape(DIM, DIM)
    wkp = wk.reshape(N_KV_HEADS, HEAD_DIM, DIM)[:, perm, :].reshape(
        N_KV_HEADS * HEAD_DIM, DIM
    )

    def sw_x(xmat):  # [T, DIM] -> [128, ND*T]
        return np.ascontiguousarray(
            xmat.T.reshape(ND, 128, T).transpose(1, 0, 2).reshape(128, ND * T)
        )

    def sw_w(wmat):  # [E(128), DIM] -> [128, ND*E]
        E = wmat.shape[0]
        return np.ascontiguousarray(
            wmat.T.reshape(ND, 128, E).transpose(1, 0, 2).reshape(128, ND * E)
        )

    xT = sw_x(x.reshape(T, DIM)).astype(xd)
    C0 = np.vstack([fc.T, fc.T])  # [128, 128]: row p -> cos[t, p % 64]
    S0 = np.vstack([-fs.T, fs.T])
    td = bf16 if fast else f32
    cq = np.ascontiguousarray(np.tile(C0 * SCALE, (1, BSZ))).astype(td)
    sq = np.ascontiguousarray(np.tile(S0 * SCALE, (1, BSZ))).astype(td)
    ck = np.ascontiguousarray(np.tile(C0, (1, BSZ))).astype(td)
    sk = np.ascontiguousarray(np.tile(S0, (1, BSZ))).astype(td)
    ident = np.eye(128, dtype=bf16)

    in_maps = []
    xj = xT.reshape(128, ND, T)
    for c in range(NCORES):
        qrows = slice(c * EQ, (c + 1) * EQ)
        krows = slice(c * HEAD_DIM, (c + 1) * HEAD_DIM)
        wq_heads = [
            sw_w(wqp[c * EQ + h * HEAD_DIM : c * EQ + (h + 1) * HEAD_DIM, :])
            for h in range(HQ)
        ]
        wo_sw = np.ascontiguousarray(
            wo[:, qrows].T.reshape(HQ, 128, DIM).transpose(1, 0, 2).reshape(128, HQ * DIM)
        ).astype(bf16)
        if fast:
            wkj = sw_w(wkp[krows, :]).reshape(128, ND, HEAD_DIM)
            wvj = sw_w(wv[krows, :]).reshape(128, ND, HEAD_DIM)
            wq0j = wq_heads[0].reshape(128, ND, HEAD_DIM)
            wq1j = wq_heads[1].reshape(128, ND, HEAD_DIM)
            sxw = np.ascontiguousarray(
                np.concatenate([xj, wkj, wvj, wq0j, wq1j], axis=2).reshape(
                    128, ND * JW
                )
            ).astype(xd)
            m = {
                "sxw": sxw,
                "wq2T": np.ascontiguousarray(wq_heads[2]).astype(xd),
                "wq3T": np.ascontiguousarray(wq_heads[3]).astype(xd),
                "woT": wo_sw,
                "cq": cq,
                "sq": sq,
                "ck": ck,
                "sk": sk,
                "ident": ident,
                "mask4": np.ascontiguousarray(np.tile(mask[0, 0], (1, BSZ))).astype(
                    bf16
                ),
            }
        else:
            m = {
                "xT": xT,
                "wqT": np.ascontiguousarray(np.concatenate(wq_heads, axis=1)).astype(
                    xd
                ),
                "wkT": sw_w(wkp[krows, :]).astype(xd),
                "wvT": sw_w(wv[krows, :]).astype(xd),
                "woT": wo_sw,
                "cq": cq,
                "sq": sq,
                "ck": ck,
                "sk": sk,
                "ident": ident,
                "mask1": np.ascontiguousarray(mask[0, 0]),
            }
        in_maps.append(m)
    return in_maps


def _pick_fast(x, wq):
    """bf16 q/k only when softmax logits are smooth (score sigma small)."""
    sx = float(np.asarray(x, np.float32).std())
    sw = float(np.asarray(wq, np.float32).std())
    sigma = sx * sw * math.sqrt(DIM * HEAD_DIM) * SCALE
    return sigma < 8.0


def kernel(
    x,
    wq,
    wk,
    wv,
    wo,
    cache_k,
    cache_v,
    freqs_cos,
    freqs_sin,
    mask,
    start_pos,
    *,
    trace=False,
    trace_kwargs=None,
):
    global LAST_RESULT
    sp = int(np.asarray(start_pos))
    assert sp == 0, f"kernel specialized for start_pos=0, got {sp}"

    fast = _pick_fast(x, wq)
    in_maps = _prep_in_maps(x, wq, wk, wv, wo, freqs_cos, freqs_sin, mask, fast)
    nc = get_nc(fast)
    res = run_bass_kernel_spmd(
        nc,
        in_maps,
        core_ids=list(range(NCORES)),
        trace=trace,
        **(trace_kwargs or {}),
    )
    LAST_RESULT = res
    acc = np.zeros((DIM, T), np.float32)
    for c in range(NCORES):
        acc += res.results[c]["yT"].astype(np.float32)
    return np.ascontiguousarray(acc.T).reshape(BSZ, SEQLEN, DIM)
